# revision 25
# baseline (speedup 1.0000x reference)
"""Distributed sparse-MoE routing kernel for 8 Trainium2 NeuronCores.

Problem (hardcoded shapes): x [4, 2048, 1024] fp32, router Wg [1024, 8],
single shared expert We [1024, 1024] + be [1024], top-1 routing with
per-expert capacity 1024 (= N/E), over-capacity tokens dropped.

The reference's dispatch/combine einsums are one-hot permutations and all
E experts apply the same (We, be), so the computation collapses exactly to

    out[n] = kept_n * gate_n * (h[n] @ We + be)

where gate_n is the top-1 softmax prob and kept_n depends on the token's
global position in its expert's queue (cumulative count in token order).

Sharding: tokens split 8 ways (1024/core); Wg/We/be replicated. Each core
routes its shard locally; the only global coupling is the per-expert
token-count prefix across cores, resolved with an 8x8-value AllGather that
overlaps the main matmul.

Per core on device:
  - router logits via a split-precision all-fp16 PE matmul: every PE
    matmul path truncates operands to ~11 mantissa bits (measured ~4e-4
    logit error for plain fp32, enough to flip argmax at observed ~1e-5
    top-2 gaps and cascade through the capacity cutoffs), so h and Wg
    are split on host: logits = h16@Wg16 + h_lo@Wg16 + 2^-12*(h16@Wg_loS)
    with h_lo = fp16(h - fp16(h)) and Wg_loS the 2^12-scaled Wg residual
    (scaling dodges fp16 subnormals), giving ~3e-7-accurate logits
  - softmax / one-hot argmax on DVE+ACT, batched in a [128, 64] layout
  - within-shard queue positions via triangular/ones bf16 PE matmuls
  - counts AllGather -> per-core prefix matmul -> kept mask + gate scale
  - main [1024x1024]@[1024x1024] matmul in fp16 (same ~11-bit operand
    precision the PE gives fp32r, ~3e-4 absmax rel err, but fast FWL
    weight loads and half the operand DMA); per-tile PSUM eviction folds
    in the bias add (DMA-broadcast be tile) and the kept*gate scale
  - the post-AllGather offset/scale chain is DVE+DMA only (broadcast
    gather + masked reduce), so the PE never blocks on the collective
"""

import numpy as np
import ml_dtypes

import concourse.bass as bass
import concourse.mybir as mybir
import concourse.tile as tile
from concourse import bacc
from concourse.bass_utils import run_bass_kernel_spmd

B, S, D = 4, 2048, 1024
E = 8
N_CORES = 8
N = B * S                  # 8192 tokens total
T = N // N_CORES           # 1024 tokens per core
CAP = N // E               # capacity per expert
P = 128
NK = D // P                # 8 contraction tiles
NM = T // P                # 8 token tiles per core
HF = 512                   # main matmul free-dim half (PSUM bank)

F32 = mybir.dt.float32
F32R = mybir.dt.float32r
BF16 = mybir.dt.bfloat16
F16 = mybir.dt.float16
ACT_COPY = mybir.ActivationFunctionType.Copy
ACT_EXP = mybir.ActivationFunctionType.Exp
ALU = mybir.AluOpType


def _build_nc() -> bass.Bass:
    nc = bacc.Bacc("TRN2", target_bir_lowering=False, debug=False,
                   enable_asserts=False, num_devices=N_CORES)

    htlo_d = nc.dram_tensor("htlo", [D, T], F16, kind="ExternalInput")
    wgp16_d = nc.dram_tensor("wgp16", [D, 2 * E], F16, kind="ExternalInput")
    ht16_d = nc.dram_tensor("ht16", [D, T], F16, kind="ExternalInput")
    we16_d = nc.dram_tensor("we16", [D, D], F16, kind="ExternalInput")
    be_d = nc.dram_tensor("be", [1, D], F32, kind="ExternalInput")
    wpre_d = nc.dram_tensor("wpre", [1, N_CORES], F32, kind="ExternalInput")
    out_d = nc.dram_tensor("out", [T, D], F32, kind="ExternalOutput")

    # Constants baked into the NEFF. tri[k, m] = 1 iff k <= m: token k
    # counts toward token m's inclusive queue position.
    tri_d = nc.inline_tensor(
        np.triu(np.ones((P, P))).astype(ml_dtypes.bfloat16), name="tri_c")
    ones_d = nc.inline_tensor(
        np.ones((P, P), dtype=ml_dtypes.bfloat16), name="ones_c")


    with tile.TileContext(nc) as tc:
        with (
            tc.tile_pool(name="const", bufs=1) as const,
            tc.tile_pool(name="htp", bufs=1) as htp,
            tc.tile_pool(name="wep", bufs=1) as wep,
            tc.tile_pool(name="small", bufs=1) as small,
            tc.tile_pool(name="psq", bufs=8, space="PSUM") as psq,
            tc.tile_pool(name="outp", bufs=1) as outp,
            tc.tile_pool(name="dram", bufs=1, space="DRAM") as dram,
        ):
            # ---- loads, ordered for the critical path: the router needs
            # ht k-tiles + wg first; We/constants can trickle in behind. ----
            htlo_sb = htp.tile([P, NK * T], F16, tag="htlo")
            ht16_sb = htp.tile([P, NK * T], F16, tag="ht16")
            we16_sb = wep.tile([P, NK * D], F16, tag="we16")
            wgp16_sb = const.tile([P, NK * 2 * E], F16, tag="wgp16")
            tri_sb = const.tile([P, P], BF16, tag="tri")
            ones_sb = const.tile([P, P], BF16, tag="ones")
            be_bc = wep.tile([P, D], F32, tag="be_bc")
            wpre_bc = const.tile([P, N_CORES], F32, tag="wpre")

            def load_ht(k, h):
                # half h of k-tile: tokens [h*512, h*512+512) = router group h
                nc.sync.dma_start(
                    ht16_sb[:, k * T + h * (T // 2): k * T + (h + 1) * (T // 2)],
                    ht16_d[k * P:(k + 1) * P, h * (T // 2):(h + 1) * (T // 2)])
                nc.sync.dma_start(
                    htlo_sb[:, k * T + h * (T // 2): k * T + (h + 1) * (T // 2)],
                    htlo_d[k * P:(k + 1) * P, h * (T // 2):(h + 1) * (T // 2)])

            nc.sync.dma_start(
                wgp16_sb[:].rearrange("p (k e) -> p k e", e=2 * E),
                wgp16_d[:, :].rearrange("(k p) e -> p k e", p=P))
            for k in range(NK):
                load_ht(k, 0)
            for k in range(NK):
                nc.sync.dma_start(
                    we16_sb[:, k * D:(k + 1) * D], we16_d[k * P:(k + 1) * P, :])
            bev = be_d[:, :]
            nc.sync.dma_start(
                be_bc[:], bass.AP(bev.tensor, bev.offset,
                                  [[0, P], [1, D]]))
            for k in range(NK):
                load_ht(k, 1)
            nc.sync.dma_start(ones_sb[:], ones_d[:, :])
            nc.sync.dma_start(tri_sb[:], tri_d[:, :])
            wpv = wpre_d[:, :]
            nc.sync.dma_start(
                wpre_bc[:], bass.AP(wpv.tensor, wpv.offset,
                                    [[0, P], [1, N_CORES]]))

            # ---- router logits, split-precision fp16 ----
            # k-outer in two groups of 4 token tiles so PE starts as soon
            # as the first ht k-tile DMA lands; main tiles b0..b2 are
            # interleaved after group A to fill the group-B load shadow.
            # logits = h16@Wg16 + h_lo@Wg16 + 2^-12*(h16@Wg_loS): operands
            # exactly representable in fp16 (host pre-split), ~3e-7 logits.
            logits_all = small.tile([P, NM * E], F32, tag="logits")

            def mm_tile(b):
                pm0 = psq.tile([P, HF], F32, tag="ps", name=f"pm0_{b}")
                pm1 = psq.tile([P, HF], F32, tag="ps", name=f"pm1_{b}")
                for half, pm in ((0, pm0), (1, pm1)):
                    for k in range(NK):
                        nc.tensor.matmul(
                            pm[:],
                            ht16_sb[:, k * T + b * P: k * T + (b + 1) * P],
                            we16_sb[:, k * D + half * HF: k * D + (half + 1) * HF],
                            start=(k == 0), stop=(k == NK - 1))
                return pm0, pm1

            def bias_evict(b, pm0, pm1):
                # AG-independent psum eviction with the bias folded in
                ot = outp.tile([P, D], F32, tag=f"ot{b}", name=f"ot{b}")
                nc.vector.tensor_tensor(
                    ot[:, 0:HF], pm0[:], be_bc[:, 0:HF], ALU.add)
                nc.vector.tensor_tensor(
                    ot[:, HF:D], pm1[:], be_bc[:, HF:D], ALU.add)
                return ot

            def store(b, ot):
                # in-place gate*kept scale; ship each half as soon as ready
                sc = scale_all[:, b:b + 1]
                nc.vector.tensor_scalar(
                    ot[:, 0:HF], ot[:, 0:HF], sc, None, ALU.mult)
                nc.sync.dma_start(out_d[b * P:(b + 1) * P, 0:HF], ot[:, 0:HF])
                nc.scalar.activation(
                    ot[:, HF:D], ot[:, HF:D], ACT_COPY, scale=sc)
                nc.sync.dma_start(out_d[b * P:(b + 1) * P, HF:D], ot[:, HF:D])

            N_EARLY = 3   # main tiles run inside the group-B load shadow
            early = {}
            for g in range(2):
                pls = [psq.tile([P, 2 * E], F32, tag="ps", name=f"pl{g}_{i}")
                       for i in range(4)]
                for k in range(NK):
                    for i, pl in enumerate(pls):
                        b = g * 4 + i
                        hi = ht16_sb[:, k * T + b * P: k * T + (b + 1) * P]
                        lo = htlo_sb[:, k * T + b * P: k * T + (b + 1) * P]
                        nc.tensor.matmul(
                            pl[:], hi, wgp16_sb[:, k * 2 * E:(k + 1) * 2 * E],
                            start=(k == 0), stop=False,
                            skip_group_check=True)
                        nc.tensor.matmul(
                            pl[:, 0:E], lo,
                            wgp16_sb[:, k * 2 * E: k * 2 * E + E],
                            start=False, stop=(k == NK - 1),
                            skip_group_check=True)
                for i, pl in enumerate(pls):
                    b = g * 4 + i
                    # logits = (hi@Wg_hi + lo@Wg_hi) + 2^-12 * (hi@Wg_loS)
                    la_sb = small.tile([P, E], F32, tag="lA", name=f"lA{g}_{i}")
                    nc.scalar.activation(la_sb[:], pl[:, 0:E], ACT_COPY)
                    nc.vector.scalar_tensor_tensor(
                        logits_all[:, b * E:(b + 1) * E],
                        pl[:, E:2 * E], 1.0 / 4096.0, la_sb[:],
                        ALU.mult, ALU.add)
                if g == 0:
                    for b in range(N_EARLY):
                        pms = mm_tile(b)
                        early[b] = bias_evict(b, *pms)

            # ---- softmax / one-hot argmax, batched over all token tiles ----
            la = logits_all[:]
            l3 = la.rearrange("p (b e) -> p b e", e=E)
            lmax = small.tile([P, NM], F32, tag="lmax")
            nc.vector.tensor_reduce(lmax[:], l3, mybir.AxisListType.X, ALU.max)
            lm = lmax[:]
            lmax_b = bass.AP(lm.tensor, lm.offset, [lm.ap[0], [1, NM], [0, E]])
            lsub = small.tile([P, NM * E], F32, tag="lsub")
            nc.vector.tensor_tensor(
                lsub[:].rearrange("p (b e) -> p b e", e=E), l3, lmax_b,
                ALU.subtract)
            expd = small.tile([P, NM * E], F32, tag="expd")
            nc.scalar.activation(expd[:], lsub[:], ACT_EXP)
            ssum = small.tile([P, NM], F32, tag="ssum")
            nc.vector.tensor_reduce(
                ssum[:], expd[:].rearrange("p (b e) -> p b e", e=E),
                mybir.AxisListType.X, ALU.add)
            gate = small.tile([P, NM], F32, tag="gate")
            nc.vector.reciprocal(gate[:], ssum[:])
            mask_all = small.tile([P, NM * E], BF16, tag="mask")
            nc.vector.tensor_tensor(
                mask_all[:].rearrange("p (b e) -> p b e", e=E), l3, lmax_b,
                ALU.is_equal)

            # ---- per-core expert counts, AllGather ASAP ----
            # counts = sum_b ones.T @ mask_b (every output row holds the
            # count); launched before the loc matmuls so the collective
            # overlaps them and the main matmul.
            pcnt = psq.tile([P, E], F32, tag="ps")
            for b in range(NM):
                nc.tensor.matmul(
                    pcnt[:], ones_sb[:], mask_all[:, b * E:(b + 1) * E],
                    start=(b == 0), stop=(b == NM - 1))
            cnt_sb = small.tile([1, E], F32, tag="cnt")
            nc.scalar.activation(cnt_sb[:], pcnt[0:1, :], ACT_COPY)
            ag_in = dram.tile([1, E], F32)
            ag_out = dram.tile([N_CORES, E], F32, addr_space="Shared")
            nc.gpsimd.dma_start(ag_in[:], cnt_sb[:])
            nc.gpsimd.collective_compute(
                "AllGather", ALU.bypass,
                ins=[ag_in[:].opt()],
                outs=[ag_out[:].opt()],
                replica_groups=[list(range(N_CORES))])
            agout_bc = small.tile([P, N_CORES * E], F32, tag="agout")
            agv = ag_out[:]
            nc.gpsimd.dma_start(
                agout_bc[:], bass.AP(agv.tensor, agv.offset,
                                     [[0, P], [1, N_CORES * E]]))

            # ---- within-shard inclusive queue positions ----
            loc_all = small.tile([P, NM * E], F32, tag="loc")
            for b in range(NM):
                ploc = psq.tile([P, E], F32, tag="ps")
                nc.tensor.matmul(
                    ploc[:], tri_sb[:], mask_all[:, b * E:(b + 1) * E],
                    start=True, stop=(b == 0))
                for a in range(b):
                    nc.tensor.matmul(
                        ploc[:], ones_sb[:], mask_all[:, a * E:(a + 1) * E],
                        start=False, stop=(a == b - 1))
                nc.scalar.activation(
                    loc_all[:, b * E:(b + 1) * E], ploc[:], ACT_COPY)

            # ---- offsets + per-token scale: DVE-only, so the PE never
            # waits on the collective; runs as soon as the AllGather lands ----
            offs_sb = small.tile([P, E], F32, tag="offs")
            scale_all = small.tile([P, NM], F32, tag="scale")

            def scale_chain():
                ag3 = agout_bc[:].rearrange("p (c e) -> p c e", e=E)
                wp = wpre_bc[:]
                wp3 = bass.AP(wp.tensor, wp.offset,
                              [wp.ap[0], [1, N_CORES], [0, E]])
                agm = small.tile([P, N_CORES * E], F32, tag="agm")
                nc.vector.tensor_tensor(
                    agm[:].rearrange("p (c e) -> p c e", e=E), ag3, wp3,
                    ALU.mult)
                am = agm[:]
                nc.vector.tensor_reduce(
                    offs_sb[:],
                    bass.AP(am.tensor, am.offset,
                            [am.ap[0], [1, E], [E, N_CORES]]),
                    mybir.AxisListType.X, ALU.add)
                of = offs_sb[:]
                offs_b = bass.AP(
                    of.tensor, of.offset, [of.ap[0], [0, NM], [1, E]])
                locg = small.tile([P, NM * E], F32, tag="locg")
                nc.vector.tensor_tensor(
                    locg[:].rearrange("p (b e) -> p b e", e=E),
                    loc_all[:].rearrange("p (b e) -> p b e", e=E),
                    offs_b, ALU.add)
                kept = small.tile([P, NM * E], F32, tag="kept")
                nc.vector.tensor_scalar(
                    kept[:], locg[:], float(CAP) + 0.5, None, ALU.is_le)
                keptm = small.tile([P, NM * E], F32, tag="keptm")
                nc.vector.tensor_tensor(
                    keptm[:], kept[:], mask_all[:], ALU.mult)
                kflag = small.tile([P, NM], F32, tag="kflag")
                nc.vector.tensor_reduce(
                    kflag[:], keptm[:].rearrange("p (b e) -> p b e", e=E),
                    mybir.AxisListType.X, ALU.add)
                nc.vector.tensor_tensor(
                    scale_all[:], kflag[:], gate[:], ALU.mult)

            def scaled_evict(b, pm0, pm1):
                ot = outp.tile([P, D], F32, tag=f"ot{b}", name=f"ot{b}")
                sc = scale_all[:, b:b + 1]
                nc.vector.tensor_tensor(
                    ot[:, 0:HF], pm0[:], be_bc[:, 0:HF], ALU.add)
                nc.vector.tensor_scalar(
                    ot[:, 0:HF], ot[:, 0:HF], sc, None, ALU.mult)
                nc.sync.dma_start(out_d[b * P:(b + 1) * P, 0:HF], ot[:, 0:HF])
                nc.vector.tensor_tensor(
                    ot[:, HF:D], pm1[:], be_bc[:, HF:D], ALU.add)
                nc.scalar.activation(
                    ot[:, HF:D], ot[:, HF:D], ACT_COPY, scale=sc)
                nc.sync.dma_start(out_d[b * P:(b + 1) * P, HF:D], ot[:, HF:D])
                return ot

            scale_chain()
            for b in range(N_EARLY):
                store(b, early.pop(b))
            for b in range(N_EARLY, NM):
                pm0, pm1 = mm_tile(b)
                scaled_evict(b, pm0, pm1)

    nc.finalize()
    return nc


_NC_CACHE = None


def kernel(x: np.ndarray, Wg: np.ndarray, We: np.ndarray,
           be: np.ndarray) -> np.ndarray:
    global _NC_CACHE
    if _NC_CACHE is None:
        _NC_CACHE = _build_nc()
    nc = _NC_CACHE

    h = np.ascontiguousarray(np.asarray(x, dtype=np.float32).reshape(N, D))
    Wg = np.ascontiguousarray(np.asarray(Wg, dtype=np.float32))
    We = np.ascontiguousarray(np.asarray(We, dtype=np.float32))
    be2 = np.ascontiguousarray(np.asarray(be, dtype=np.float32).reshape(1, D))

    hT = np.ascontiguousarray(h.T)
    ht16 = hT.astype(np.float16)
    ht_lo = (hT - ht16.astype(np.float32)).astype(np.float16)
    Wg16 = Wg.astype(np.float16)
    Wg_loS = ((Wg - Wg16.astype(np.float32)) * 4096.0).astype(np.float16)
    Wgp16 = np.ascontiguousarray(np.concatenate([Wg16, Wg_loS], axis=1))
    We16 = We.astype(np.float16)

    in_maps = []
    for c in range(N_CORES):
        wpre = np.zeros((1, N_CORES), np.float32)
        wpre[0, :c] = 1.0
        in_maps.append({
            "htlo": np.ascontiguousarray(ht_lo[:, c * T:(c + 1) * T]),
            "wgp16": Wgp16,
            "ht16": np.ascontiguousarray(ht16[:, c * T:(c + 1) * T]),
            "we16": We16,
            "be": be2,
            "wpre": wpre,
        })

    res = run_bass_kernel_spmd(nc, in_maps, core_ids=list(range(N_CORES)))
    out = np.concatenate(
        [res.results[c]["out"] for c in range(N_CORES)], axis=0)
    return out.reshape(B, S, D).astype(np.float32)



# revision 26
# speedup vs baseline: 1.1169x; 1.1169x over previous
"""Distributed sparse-MoE routing kernel for 8 Trainium2 NeuronCores.

Problem (hardcoded shapes): x [4, 2048, 1024] fp32, router Wg [1024, 8],
single shared expert We [1024, 1024] + be [1024], top-1 routing with
per-expert capacity 1024 (= N/E), over-capacity tokens dropped.

The reference's dispatch/combine einsums are one-hot permutations and all
E experts apply the same (We, be), so the computation collapses exactly to

    out[n] = kept_n * gate_n * (h[n] @ We + be)

where gate_n is the top-1 softmax prob and kept_n depends on the token's
global position in its expert's queue (cumulative count in token order).

Sharding: tokens split 8 ways (1024/core); Wg/We/be replicated. Each core
routes its shard locally; the only global coupling is the per-expert
token-count prefix across cores, resolved with an 8x8-value AllGather that
overlaps the main matmul.

Per core on device:
  - router logits via a split-precision all-fp16 PE matmul: every PE
    matmul path truncates operands to ~11 mantissa bits (measured ~4e-4
    logit error for plain fp32, enough to flip argmax at observed ~1e-5
    top-2 gaps and cascade through the capacity cutoffs), so h and Wg
    are split on host: logits = h16@Wg16 + h_lo@Wg16 + 2^-12*(h16@Wg_loS)
    with h_lo = fp16(h - fp16(h)) and Wg_loS the 2^12-scaled Wg residual
    (scaling dodges fp16 subnormals), giving ~3e-7-accurate logits
  - softmax / one-hot argmax on DVE+ACT, batched in a [128, 64] layout
  - within-shard queue positions via triangular/ones bf16 PE matmuls
  - counts AllGather -> per-core prefix matmul -> kept mask + gate scale
  - main [1024x1024]@[1024x1024] matmul in fp16 (same ~11-bit operand
    precision the PE gives fp32r, ~3e-4 absmax rel err, but fast FWL
    weight loads and half the operand DMA); per-tile PSUM eviction folds
    in the bias add (DMA-broadcast be tile) and the kept*gate scale
  - the post-AllGather offset/scale chain is DVE+DMA only (broadcast
    gather + masked reduce), so the PE never blocks on the collective
"""

import numpy as np
import ml_dtypes

import concourse.bass as bass
import concourse.mybir as mybir
import concourse.tile as tile
from concourse import bacc
from concourse.bass_utils import run_bass_kernel_spmd

B, S, D = 4, 2048, 1024
E = 8
N_CORES = 8
N = B * S                  # 8192 tokens total
T = N // N_CORES           # 1024 tokens per core
CAP = N // E               # capacity per expert
P = 128
NK = D // P                # 8 contraction tiles
NM = T // P                # 8 token tiles per core
HF = 512                   # main matmul free-dim half (PSUM bank)

F32 = mybir.dt.float32
F32R = mybir.dt.float32r
BF16 = mybir.dt.bfloat16
F16 = mybir.dt.float16
ACT_COPY = mybir.ActivationFunctionType.Copy
ACT_EXP = mybir.ActivationFunctionType.Exp
ALU = mybir.AluOpType


def _build_nc() -> bass.Bass:
    nc = bacc.Bacc("TRN2", target_bir_lowering=False, debug=False,
                   enable_asserts=False, num_devices=N_CORES)

    htlo_d = nc.dram_tensor("htlo", [D, T], F16, kind="ExternalInput")
    wgp16_d = nc.dram_tensor("wgp16", [D, 2 * E], F16, kind="ExternalInput")
    ht16_d = nc.dram_tensor("ht16", [D, T], F16, kind="ExternalInput")
    we16_d = nc.dram_tensor("we16", [D, D], F16, kind="ExternalInput")
    be_d = nc.dram_tensor("be", [1, D], F32, kind="ExternalInput")
    wpre_d = nc.dram_tensor("wpre", [1, N_CORES], F32, kind="ExternalInput")
    out_d = nc.dram_tensor("out", [T, D], F32, kind="ExternalOutput")

    # Constants baked into the NEFF. tri[k, m] = 1 iff k <= m: token k
    # counts toward token m's inclusive queue position.
    tri_d = nc.inline_tensor(
        np.triu(np.ones((P, P))).astype(ml_dtypes.bfloat16), name="tri_c")
    ones_d = nc.inline_tensor(
        np.ones((P, P), dtype=ml_dtypes.bfloat16), name="ones_c")


    with tile.TileContext(nc) as tc:
        with (
            tc.tile_pool(name="const", bufs=1) as const,
            tc.tile_pool(name="htp", bufs=1) as htp,
            tc.tile_pool(name="wep", bufs=1) as wep,
            tc.tile_pool(name="small", bufs=1) as small,
            tc.tile_pool(name="psq", bufs=8, space="PSUM") as psq,
            tc.tile_pool(name="outp", bufs=1) as outp,
            tc.tile_pool(name="dram", bufs=1, space="DRAM") as dram,
        ):
            # ---- loads, ordered for the critical path: the router needs
            # ht k-tiles + wg first; We/constants can trickle in behind. ----
            htlo_sb = htp.tile([P, NK * T], F16, tag="htlo")
            ht16_sb = htp.tile([P, NK * T], F16, tag="ht16")
            we16_sb = wep.tile([P, NK * D], F16, tag="we16")
            wgp16_sb = const.tile([P, NK * 2 * E], F16, tag="wgp16")
            tri_sb = const.tile([P, P], BF16, tag="tri")
            ones_sb = const.tile([P, P], BF16, tag="ones")
            be_bc = wep.tile([P, D], F32, tag="be_bc")
            wpre_bc = const.tile([P, N_CORES], F32, tag="wpre")

            def load_ht(h):
                # merged over all k-tiles: tokens [h*512, (h+1)*512)
                lo, hi = h * (T // 2), (h + 1) * (T // 2)
                nc.sync.dma_start(
                    ht16_sb[:].rearrange("p (k t) -> p k t", t=T)[:, :, lo:hi],
                    ht16_d[:, lo:hi].rearrange("(k p) t -> p k t", p=P))
                nc.sync.dma_start(
                    htlo_sb[:].rearrange("p (k t) -> p k t", t=T)[:, :, lo:hi],
                    htlo_d[:, lo:hi].rearrange("(k p) t -> p k t", p=P))

            def load_we(k0, k1):
                nc.sync.dma_start(
                    we16_sb[:].rearrange("p (k d) -> p k d", d=D)[:, k0:k1, :],
                    we16_d[k0 * P:k1 * P, :].rearrange("(k p) d -> p k d", p=P))

            nc.sync.dma_start(
                wgp16_sb[:].rearrange("p (k e) -> p k e", e=2 * E),
                wgp16_d[:, :].rearrange("(k p) e -> p k e", p=P))
            load_ht(0)
            load_we(0, 4)
            bev = be_d[:, :]
            nc.sync.dma_start(
                be_bc[:], bass.AP(bev.tensor, bev.offset,
                                  [[0, P], [1, D]]))
            load_ht(1)
            load_we(4, 8)
            nc.sync.dma_start(ones_sb[:], ones_d[:, :])
            nc.sync.dma_start(tri_sb[:], tri_d[:, :])
            wpv = wpre_d[:, :]
            nc.sync.dma_start(
                wpre_bc[:], bass.AP(wpv.tensor, wpv.offset,
                                    [[0, P], [1, N_CORES]]))

            # ---- router logits, split-precision fp16 ----
            # k-outer in two groups of 4 token tiles so PE starts as soon
            # as the first ht k-tile DMA lands; main tiles b0..b2 are
            # interleaved after group A to fill the group-B load shadow.
            # logits = h16@Wg16 + h_lo@Wg16 + 2^-12*(h16@Wg_loS): operands
            # exactly representable in fp16 (host pre-split), ~3e-7 logits.
            logits_all = small.tile([P, NM * E], F32, tag="logits")

            def mm_tile(b):
                pm0 = psq.tile([P, HF], F32, tag="ps", name=f"pm0_{b}")
                pm1 = psq.tile([P, HF], F32, tag="ps", name=f"pm1_{b}")
                for half, pm in ((0, pm0), (1, pm1)):
                    for k in range(NK):
                        nc.tensor.matmul(
                            pm[:],
                            ht16_sb[:, k * T + b * P: k * T + (b + 1) * P],
                            we16_sb[:, k * D + half * HF: k * D + (half + 1) * HF],
                            start=(k == 0), stop=(k == NK - 1))
                return pm0, pm1

            def bias_evict(b, pm0, pm1):
                # AG-independent psum eviction with the bias folded in
                ot = outp.tile([P, D], F32, tag=f"ot{b}", name=f"ot{b}")
                nc.vector.tensor_tensor(
                    ot[:, 0:HF], pm0[:], be_bc[:, 0:HF], ALU.add)
                nc.vector.tensor_tensor(
                    ot[:, HF:D], pm1[:], be_bc[:, HF:D], ALU.add)
                return ot

            def store(b, ot):
                # in-place gate*kept scale; ship each half as soon as ready
                sc = scale_all[:, b:b + 1]
                nc.vector.tensor_scalar(
                    ot[:, 0:HF], ot[:, 0:HF], sc, None, ALU.mult)
                nc.sync.dma_start(out_d[b * P:(b + 1) * P, 0:HF], ot[:, 0:HF])
                nc.scalar.activation(
                    ot[:, HF:D], ot[:, HF:D], ACT_COPY, scale=sc)
                nc.sync.dma_start(out_d[b * P:(b + 1) * P, HF:D], ot[:, HF:D])

            N_EARLY = 3   # main tiles run inside the group-B load shadow
            early = {}
            for g in range(2):
                pls = [psq.tile([P, 2 * E], F32, tag="ps", name=f"pl{g}_{i}")
                       for i in range(4)]
                for k in range(NK):
                    for i, pl in enumerate(pls):
                        b = g * 4 + i
                        hi = ht16_sb[:, k * T + b * P: k * T + (b + 1) * P]
                        lo = htlo_sb[:, k * T + b * P: k * T + (b + 1) * P]
                        nc.tensor.matmul(
                            pl[:], hi, wgp16_sb[:, k * 2 * E:(k + 1) * 2 * E],
                            start=(k == 0), stop=False,
                            skip_group_check=True)
                        nc.tensor.matmul(
                            pl[:, 0:E], lo,
                            wgp16_sb[:, k * 2 * E: k * 2 * E + E],
                            start=False, stop=(k == NK - 1),
                            skip_group_check=True)
                for i, pl in enumerate(pls):
                    b = g * 4 + i
                    # logits = (hi@Wg_hi + lo@Wg_hi) + 2^-12 * (hi@Wg_loS)
                    la_sb = small.tile([P, E], F32, tag="lA", name=f"lA{g}_{i}")
                    nc.scalar.activation(la_sb[:], pl[:, 0:E], ACT_COPY)
                    nc.vector.scalar_tensor_tensor(
                        logits_all[:, b * E:(b + 1) * E],
                        pl[:, E:2 * E], 1.0 / 4096.0, la_sb[:],
                        ALU.mult, ALU.add)
                if g == 0:
                    for b in range(N_EARLY):
                        pms = mm_tile(b)
                        early[b] = bias_evict(b, *pms)

            # ---- softmax / one-hot argmax, batched over all token tiles ----
            la = logits_all[:]
            l3 = la.rearrange("p (b e) -> p b e", e=E)
            lmax = small.tile([P, NM], F32, tag="lmax")
            nc.vector.tensor_reduce(lmax[:], l3, mybir.AxisListType.X, ALU.max)
            lm = lmax[:]
            lmax_b = bass.AP(lm.tensor, lm.offset, [lm.ap[0], [1, NM], [0, E]])
            lsub = small.tile([P, NM * E], F32, tag="lsub")
            nc.vector.tensor_tensor(
                lsub[:].rearrange("p (b e) -> p b e", e=E), l3, lmax_b,
                ALU.subtract)
            expd = small.tile([P, NM * E], F32, tag="expd")
            nc.scalar.activation(expd[:], lsub[:], ACT_EXP)
            ssum = small.tile([P, NM], F32, tag="ssum")
            nc.vector.tensor_reduce(
                ssum[:], expd[:].rearrange("p (b e) -> p b e", e=E),
                mybir.AxisListType.X, ALU.add)
            gate = small.tile([P, NM], F32, tag="gate")
            nc.vector.reciprocal(gate[:], ssum[:])
            mask_all = small.tile([P, NM * E], BF16, tag="mask")
            nc.vector.tensor_tensor(
                mask_all[:].rearrange("p (b e) -> p b e", e=E), l3, lmax_b,
                ALU.is_equal)

            # ---- per-core expert counts, AllGather ASAP ----
            # counts = sum_b ones.T @ mask_b (every output row holds the
            # count); launched before the loc matmuls so the collective
            # overlaps them and the main matmul.
            pcnt = psq.tile([P, E], F32, tag="ps")
            for b in range(NM):
                nc.tensor.matmul(
                    pcnt[:], ones_sb[:], mask_all[:, b * E:(b + 1) * E],
                    start=(b == 0), stop=(b == NM - 1))
            cnt_sb = small.tile([1, E], F32, tag="cnt")
            nc.scalar.activation(cnt_sb[:], pcnt[0:1, :], ACT_COPY)
            ag_in = dram.tile([1, E], F32)
            ag_out = dram.tile([N_CORES, E], F32, addr_space="Shared")
            nc.gpsimd.dma_start(ag_in[:], cnt_sb[:])
            nc.gpsimd.collective_compute(
                "AllGather", ALU.bypass,
                ins=[ag_in[:].opt()],
                outs=[ag_out[:].opt()],
                replica_groups=[list(range(N_CORES))])
            agout_bc = small.tile([P, N_CORES * E], F32, tag="agout")
            agv = ag_out[:]
            nc.gpsimd.dma_start(
                agout_bc[:], bass.AP(agv.tensor, agv.offset,
                                     [[0, P], [1, N_CORES * E]]))

            # ---- within-shard inclusive queue positions ----
            loc_all = small.tile([P, NM * E], F32, tag="loc")
            for b in range(NM):
                ploc = psq.tile([P, E], F32, tag="ps")
                nc.tensor.matmul(
                    ploc[:], tri_sb[:], mask_all[:, b * E:(b + 1) * E],
                    start=True, stop=(b == 0))
                for a in range(b):
                    nc.tensor.matmul(
                        ploc[:], ones_sb[:], mask_all[:, a * E:(a + 1) * E],
                        start=False, stop=(a == b - 1))
                nc.scalar.activation(
                    loc_all[:, b * E:(b + 1) * E], ploc[:], ACT_COPY)

            # ---- offsets + per-token scale: DVE-only, so the PE never
            # waits on the collective; runs as soon as the AllGather lands ----
            offs_sb = small.tile([P, E], F32, tag="offs")
            scale_all = small.tile([P, NM], F32, tag="scale")

            def scale_chain():
                ag3 = agout_bc[:].rearrange("p (c e) -> p c e", e=E)
                wp = wpre_bc[:]
                wp3 = bass.AP(wp.tensor, wp.offset,
                              [wp.ap[0], [1, N_CORES], [0, E]])
                agm = small.tile([P, N_CORES * E], F32, tag="agm")
                nc.vector.tensor_tensor(
                    agm[:].rearrange("p (c e) -> p c e", e=E), ag3, wp3,
                    ALU.mult)
                am = agm[:]
                nc.vector.tensor_reduce(
                    offs_sb[:],
                    bass.AP(am.tensor, am.offset,
                            [am.ap[0], [1, E], [E, N_CORES]]),
                    mybir.AxisListType.X, ALU.add)
                of = offs_sb[:]
                offs_b = bass.AP(
                    of.tensor, of.offset, [of.ap[0], [0, NM], [1, E]])
                locg = small.tile([P, NM * E], F32, tag="locg")
                nc.vector.tensor_tensor(
                    locg[:].rearrange("p (b e) -> p b e", e=E),
                    loc_all[:].rearrange("p (b e) -> p b e", e=E),
                    offs_b, ALU.add)
                kept = small.tile([P, NM * E], F32, tag="kept")
                nc.vector.tensor_scalar(
                    kept[:], locg[:], float(CAP) + 0.5, None, ALU.is_le)
                keptm = small.tile([P, NM * E], F32, tag="keptm")
                nc.vector.tensor_tensor(
                    keptm[:], kept[:], mask_all[:], ALU.mult)
                kflag = small.tile([P, NM], F32, tag="kflag")
                nc.vector.tensor_reduce(
                    kflag[:], keptm[:].rearrange("p (b e) -> p b e", e=E),
                    mybir.AxisListType.X, ALU.add)
                nc.vector.tensor_tensor(
                    scale_all[:], kflag[:], gate[:], ALU.mult)

            def scaled_evict(b, pm0, pm1):
                ot = outp.tile([P, D], F32, tag=f"ot{b}", name=f"ot{b}")
                sc = scale_all[:, b:b + 1]
                nc.vector.tensor_tensor(
                    ot[:, 0:HF], pm0[:], be_bc[:, 0:HF], ALU.add)
                nc.vector.tensor_scalar(
                    ot[:, 0:HF], ot[:, 0:HF], sc, None, ALU.mult)
                nc.sync.dma_start(out_d[b * P:(b + 1) * P, 0:HF], ot[:, 0:HF])
                nc.vector.tensor_tensor(
                    ot[:, HF:D], pm1[:], be_bc[:, HF:D], ALU.add)
                nc.scalar.activation(
                    ot[:, HF:D], ot[:, HF:D], ACT_COPY, scale=sc)
                nc.sync.dma_start(out_d[b * P:(b + 1) * P, HF:D], ot[:, HF:D])
                return ot

            scale_chain()
            for b in range(N_EARLY):
                store(b, early.pop(b))
            for b in range(N_EARLY, NM):
                pm0, pm1 = mm_tile(b)
                scaled_evict(b, pm0, pm1)

    nc.finalize()
    return nc


_NC_CACHE = None


def kernel(x: np.ndarray, Wg: np.ndarray, We: np.ndarray,
           be: np.ndarray) -> np.ndarray:
    global _NC_CACHE
    if _NC_CACHE is None:
        _NC_CACHE = _build_nc()
    nc = _NC_CACHE

    h = np.ascontiguousarray(np.asarray(x, dtype=np.float32).reshape(N, D))
    Wg = np.ascontiguousarray(np.asarray(Wg, dtype=np.float32))
    We = np.ascontiguousarray(np.asarray(We, dtype=np.float32))
    be2 = np.ascontiguousarray(np.asarray(be, dtype=np.float32).reshape(1, D))

    hT = np.ascontiguousarray(h.T)
    ht16 = hT.astype(np.float16)
    ht_lo = (hT - ht16.astype(np.float32)).astype(np.float16)
    Wg16 = Wg.astype(np.float16)
    Wg_loS = ((Wg - Wg16.astype(np.float32)) * 4096.0).astype(np.float16)
    Wgp16 = np.ascontiguousarray(np.concatenate([Wg16, Wg_loS], axis=1))
    We16 = We.astype(np.float16)

    in_maps = []
    for c in range(N_CORES):
        wpre = np.zeros((1, N_CORES), np.float32)
        wpre[0, :c] = 1.0
        in_maps.append({
            "htlo": np.ascontiguousarray(ht_lo[:, c * T:(c + 1) * T]),
            "wgp16": Wgp16,
            "ht16": np.ascontiguousarray(ht16[:, c * T:(c + 1) * T]),
            "we16": We16,
            "be": be2,
            "wpre": wpre,
        })

    res = run_bass_kernel_spmd(nc, in_maps, core_ids=list(range(N_CORES)))
    out = np.concatenate(
        [res.results[c]["out"] for c in range(N_CORES)], axis=0)
    return out.reshape(B, S, D).astype(np.float32)



# revision 27
# speedup vs baseline: 1.1303x; 1.0120x over previous
"""Distributed sparse-MoE routing kernel for 8 Trainium2 NeuronCores.

Problem (hardcoded shapes): x [4, 2048, 1024] fp32, router Wg [1024, 8],
single shared expert We [1024, 1024] + be [1024], top-1 routing with
per-expert capacity 1024 (= N/E), over-capacity tokens dropped.

The reference's dispatch/combine einsums are one-hot permutations and all
E experts apply the same (We, be), so the computation collapses exactly to

    out[n] = kept_n * gate_n * (h[n] @ We + be)

where gate_n is the top-1 softmax prob and kept_n depends on the token's
global position in its expert's queue (cumulative count in token order).

Sharding: tokens split 8 ways (1024/core); Wg/We/be replicated. Each core
routes its shard locally; the only global coupling is the per-expert
token-count prefix across cores, resolved with an 8x8-value AllGather that
overlaps the main matmul.

Per core on device:
  - router logits via a split-precision all-fp16 PE matmul: every PE
    matmul path truncates operands to ~11 mantissa bits (measured ~4e-4
    logit error for plain fp32, enough to flip argmax at observed ~1e-5
    top-2 gaps and cascade through the capacity cutoffs), so h and Wg
    are split on host: logits = h16@Wg16 + h_lo@Wg16 + 2^-12*(h16@Wg_loS)
    with h_lo = fp16(h - fp16(h)) and Wg_loS the 2^12-scaled Wg residual
    (scaling dodges fp16 subnormals), giving ~3e-7-accurate logits
  - softmax / one-hot argmax on DVE+ACT, batched in a [128, 64] layout
  - within-shard queue positions via triangular/ones bf16 PE matmuls
  - counts AllGather -> per-core prefix matmul -> kept mask + gate scale
  - main [1024x1024]@[1024x1024] matmul in fp16 (same ~11-bit operand
    precision the PE gives fp32r, ~3e-4 absmax rel err, but fast FWL
    weight loads and half the operand DMA); per-tile PSUM eviction folds
    in the bias add (DMA-broadcast be tile) and the kept*gate scale
  - the post-AllGather offset/scale chain is DVE+DMA only (broadcast
    gather + masked reduce), so the PE never blocks on the collective
"""

import numpy as np
import ml_dtypes

import concourse.bass as bass
import concourse.mybir as mybir
import concourse.tile as tile
from concourse import bacc
from concourse.bass_utils import run_bass_kernel_spmd

B, S, D = 4, 2048, 1024
E = 8
N_CORES = 8
N = B * S                  # 8192 tokens total
T = N // N_CORES           # 1024 tokens per core
CAP = N // E               # capacity per expert
P = 128
NK = D // P                # 8 contraction tiles
NM = T // P                # 8 token tiles per core
HF = 512                   # main matmul free-dim half (PSUM bank)

F32 = mybir.dt.float32
F32R = mybir.dt.float32r
BF16 = mybir.dt.bfloat16
F16 = mybir.dt.float16
ACT_COPY = mybir.ActivationFunctionType.Copy
ACT_EXP = mybir.ActivationFunctionType.Exp
ALU = mybir.AluOpType


def _build_nc() -> bass.Bass:
    nc = bacc.Bacc("TRN2", target_bir_lowering=False, debug=False,
                   enable_asserts=False, num_devices=N_CORES)

    htlo_d = nc.dram_tensor("htlo", [D, T], F16, kind="ExternalInput")
    wgp16_d = nc.dram_tensor("wgp16", [D, 2 * E], F16, kind="ExternalInput")
    ht16_d = nc.dram_tensor("ht16", [D, T], F16, kind="ExternalInput")
    we16_d = nc.dram_tensor("we16", [D, D], F16, kind="ExternalInput")
    be_d = nc.dram_tensor("be", [1, D], F32, kind="ExternalInput")
    wpre_d = nc.dram_tensor("wpre", [1, N_CORES], F32, kind="ExternalInput")
    out_d = nc.dram_tensor("out", [T, D], F16, kind="ExternalOutput")

    # Constants baked into the NEFF. tri[k, m] = 1 iff k <= m: token k
    # counts toward token m's inclusive queue position.
    tri_d = nc.inline_tensor(
        np.triu(np.ones((P, P))).astype(ml_dtypes.bfloat16), name="tri_c")
    ones_d = nc.inline_tensor(
        np.ones((P, P), dtype=ml_dtypes.bfloat16), name="ones_c")


    with tile.TileContext(nc) as tc:
        with (
            tc.tile_pool(name="const", bufs=1) as const,
            tc.tile_pool(name="htp", bufs=1) as htp,
            tc.tile_pool(name="wep", bufs=1) as wep,
            tc.tile_pool(name="small", bufs=1) as small,
            tc.tile_pool(name="psq", bufs=8, space="PSUM") as psq,
            tc.tile_pool(name="outp", bufs=1) as outp,
            tc.tile_pool(name="dram", bufs=1, space="DRAM") as dram,
        ):
            # ---- loads, ordered for the critical path: the router needs
            # ht k-tiles + wg first; We/constants can trickle in behind. ----
            htlo_sb = htp.tile([P, NK * T], F16, tag="htlo")
            ht16_sb = htp.tile([P, NK * T], F16, tag="ht16")
            we16_sb = wep.tile([P, NK * D], F16, tag="we16")
            wgp16_sb = const.tile([P, NK * 2 * E], F16, tag="wgp16")
            tri_sb = const.tile([P, P], BF16, tag="tri")
            ones_sb = const.tile([P, P], BF16, tag="ones")
            be_bc = wep.tile([P, D], F32, tag="be_bc")
            wpre_bc = const.tile([P, N_CORES], F32, tag="wpre")

            def load_ht(h):
                # merged over all k-tiles: tokens [h*512, (h+1)*512)
                lo, hi = h * (T // 2), (h + 1) * (T // 2)
                nc.sync.dma_start(
                    ht16_sb[:].rearrange("p (k t) -> p k t", t=T)[:, :, lo:hi],
                    ht16_d[:, lo:hi].rearrange("(k p) t -> p k t", p=P))
                nc.sync.dma_start(
                    htlo_sb[:].rearrange("p (k t) -> p k t", t=T)[:, :, lo:hi],
                    htlo_d[:, lo:hi].rearrange("(k p) t -> p k t", p=P))

            def load_we(k0, k1):
                nc.sync.dma_start(
                    we16_sb[:].rearrange("p (k d) -> p k d", d=D)[:, k0:k1, :],
                    we16_d[k0 * P:k1 * P, :].rearrange("(k p) d -> p k d", p=P))

            nc.sync.dma_start(
                wgp16_sb[:].rearrange("p (k e) -> p k e", e=2 * E),
                wgp16_d[:, :].rearrange("(k p) e -> p k e", p=P))
            load_ht(0)
            load_we(0, 4)
            bev = be_d[:, :]
            nc.sync.dma_start(
                be_bc[:], bass.AP(bev.tensor, bev.offset,
                                  [[0, P], [1, D]]))
            load_ht(1)
            load_we(4, 8)
            nc.sync.dma_start(ones_sb[:], ones_d[:, :])
            nc.sync.dma_start(tri_sb[:], tri_d[:, :])
            wpv = wpre_d[:, :]
            nc.sync.dma_start(
                wpre_bc[:], bass.AP(wpv.tensor, wpv.offset,
                                    [[0, P], [1, N_CORES]]))

            # ---- router logits, split-precision fp16 ----
            # k-outer in two groups of 4 token tiles so PE starts as soon
            # as the first ht k-tile DMA lands; main tiles b0..b2 are
            # interleaved after group A to fill the group-B load shadow.
            # logits = h16@Wg16 + h_lo@Wg16 + 2^-12*(h16@Wg_loS): operands
            # exactly representable in fp16 (host pre-split), ~3e-7 logits.
            logits_all = small.tile([P, NM * E], F32, tag="logits")

            def mm_tile(b):
                pm0 = psq.tile([P, HF], F32, tag="ps", name=f"pm0_{b}")
                pm1 = psq.tile([P, HF], F32, tag="ps", name=f"pm1_{b}")
                for half, pm in ((0, pm0), (1, pm1)):
                    for k in range(NK):
                        nc.tensor.matmul(
                            pm[:],
                            ht16_sb[:, k * T + b * P: k * T + (b + 1) * P],
                            we16_sb[:, k * D + half * HF: k * D + (half + 1) * HF],
                            start=(k == 0), stop=(k == NK - 1))
                return pm0, pm1

            def bias_evict(b, pm0, pm1):
                # AG-independent psum eviction with the bias folded in
                ot = outp.tile([P, D], F32, tag=f"ot{b}", name=f"ot{b}")
                nc.vector.tensor_tensor(
                    ot[:, 0:HF], pm0[:], be_bc[:, 0:HF], ALU.add)
                nc.vector.tensor_tensor(
                    ot[:, HF:D], pm1[:], be_bc[:, HF:D], ALU.add)
                return ot

            def store(b, ot):
                # gate*kept scale fused with fp16 downcast; ship each half
                sc = scale_all[:, b:b + 1]
                st = outp.tile([P, D], F16, tag=f"st{b}", name=f"st{b}")
                nc.vector.tensor_scalar(
                    st[:, 0:HF], ot[:, 0:HF], sc, None, ALU.mult)
                nc.sync.dma_start(out_d[b * P:(b + 1) * P, 0:HF], st[:, 0:HF])
                nc.scalar.activation(
                    st[:, HF:D], ot[:, HF:D], ACT_COPY, scale=sc)
                nc.sync.dma_start(out_d[b * P:(b + 1) * P, HF:D], st[:, HF:D])

            N_EARLY = 3   # main tiles run inside the group-B load shadow
            early = {}
            for g in range(2):
                pls = [psq.tile([P, 2 * E], F32, tag="ps", name=f"pl{g}_{i}")
                       for i in range(4)]
                for k in range(NK):
                    for i, pl in enumerate(pls):
                        b = g * 4 + i
                        hi = ht16_sb[:, k * T + b * P: k * T + (b + 1) * P]
                        lo = htlo_sb[:, k * T + b * P: k * T + (b + 1) * P]
                        nc.tensor.matmul(
                            pl[:], hi, wgp16_sb[:, k * 2 * E:(k + 1) * 2 * E],
                            start=(k == 0), stop=False,
                            skip_group_check=True)
                        nc.tensor.matmul(
                            pl[:, 0:E], lo,
                            wgp16_sb[:, k * 2 * E: k * 2 * E + E],
                            start=False, stop=(k == NK - 1),
                            skip_group_check=True)
                for i, pl in enumerate(pls):
                    b = g * 4 + i
                    # logits = (hi@Wg_hi + lo@Wg_hi) + 2^-12 * (hi@Wg_loS)
                    la_sb = small.tile([P, E], F32, tag="lA", name=f"lA{g}_{i}")
                    nc.scalar.activation(la_sb[:], pl[:, 0:E], ACT_COPY)
                    nc.vector.scalar_tensor_tensor(
                        logits_all[:, b * E:(b + 1) * E],
                        pl[:, E:2 * E], 1.0 / 4096.0, la_sb[:],
                        ALU.mult, ALU.add)
                if g == 0:
                    for b in range(N_EARLY):
                        pms = mm_tile(b)
                        early[b] = bias_evict(b, *pms)

            # ---- softmax / one-hot argmax, batched over all token tiles ----
            la = logits_all[:]
            l3 = la.rearrange("p (b e) -> p b e", e=E)
            lmax = small.tile([P, NM], F32, tag="lmax")
            nc.vector.tensor_reduce(lmax[:], l3, mybir.AxisListType.X, ALU.max)
            lm = lmax[:]
            lmax_b = bass.AP(lm.tensor, lm.offset, [lm.ap[0], [1, NM], [0, E]])
            lsub = small.tile([P, NM * E], F32, tag="lsub")
            nc.vector.tensor_tensor(
                lsub[:].rearrange("p (b e) -> p b e", e=E), l3, lmax_b,
                ALU.subtract)
            expd = small.tile([P, NM * E], F32, tag="expd")
            nc.scalar.activation(expd[:], lsub[:], ACT_EXP)
            ssum = small.tile([P, NM], F32, tag="ssum")
            nc.vector.tensor_reduce(
                ssum[:], expd[:].rearrange("p (b e) -> p b e", e=E),
                mybir.AxisListType.X, ALU.add)
            gate = small.tile([P, NM], F32, tag="gate")
            nc.vector.reciprocal(gate[:], ssum[:])
            mask_all = small.tile([P, NM * E], BF16, tag="mask")
            nc.vector.tensor_tensor(
                mask_all[:].rearrange("p (b e) -> p b e", e=E), l3, lmax_b,
                ALU.is_equal)

            # ---- per-core expert counts, AllGather ASAP ----
            # counts = sum_b ones.T @ mask_b (every output row holds the
            # count); launched before the loc matmuls so the collective
            # overlaps them and the main matmul.
            pcnt = psq.tile([P, E], F32, tag="ps")
            for b in range(NM):
                nc.tensor.matmul(
                    pcnt[:], ones_sb[:], mask_all[:, b * E:(b + 1) * E],
                    start=(b == 0), stop=(b == NM - 1))
            cnt_sb = small.tile([1, E], F32, tag="cnt")
            nc.scalar.activation(cnt_sb[:], pcnt[0:1, :], ACT_COPY)
            ag_in = dram.tile([1, E], F32)
            ag_out = dram.tile([N_CORES, E], F32, addr_space="Shared")
            nc.gpsimd.dma_start(ag_in[:], cnt_sb[:])
            nc.gpsimd.collective_compute(
                "AllGather", ALU.bypass,
                ins=[ag_in[:].opt()],
                outs=[ag_out[:].opt()],
                replica_groups=[list(range(N_CORES))])
            agout_bc = small.tile([P, N_CORES * E], F32, tag="agout")
            agv = ag_out[:]
            nc.gpsimd.dma_start(
                agout_bc[:], bass.AP(agv.tensor, agv.offset,
                                     [[0, P], [1, N_CORES * E]]))

            # ---- within-shard inclusive queue positions ----
            loc_all = small.tile([P, NM * E], F32, tag="loc")
            for b in range(NM):
                ploc = psq.tile([P, E], F32, tag="ps")
                nc.tensor.matmul(
                    ploc[:], tri_sb[:], mask_all[:, b * E:(b + 1) * E],
                    start=True, stop=(b == 0))
                for a in range(b):
                    nc.tensor.matmul(
                        ploc[:], ones_sb[:], mask_all[:, a * E:(a + 1) * E],
                        start=False, stop=(a == b - 1))
                nc.scalar.activation(
                    loc_all[:, b * E:(b + 1) * E], ploc[:], ACT_COPY)

            # ---- offsets + per-token scale: DVE-only, so the PE never
            # waits on the collective; runs as soon as the AllGather lands ----
            offs_sb = small.tile([P, E], F32, tag="offs")
            scale_all = small.tile([P, NM], F32, tag="scale")

            def scale_chain():
                ag3 = agout_bc[:].rearrange("p (c e) -> p c e", e=E)
                wp = wpre_bc[:]
                wp3 = bass.AP(wp.tensor, wp.offset,
                              [wp.ap[0], [1, N_CORES], [0, E]])
                agm = small.tile([P, N_CORES * E], F32, tag="agm")
                nc.vector.tensor_tensor(
                    agm[:].rearrange("p (c e) -> p c e", e=E), ag3, wp3,
                    ALU.mult)
                am = agm[:]
                nc.vector.tensor_reduce(
                    offs_sb[:],
                    bass.AP(am.tensor, am.offset,
                            [am.ap[0], [1, E], [E, N_CORES]]),
                    mybir.AxisListType.X, ALU.add)
                of = offs_sb[:]
                offs_b = bass.AP(
                    of.tensor, of.offset, [of.ap[0], [0, NM], [1, E]])
                locg = small.tile([P, NM * E], F32, tag="locg")
                nc.vector.tensor_tensor(
                    locg[:].rearrange("p (b e) -> p b e", e=E),
                    loc_all[:].rearrange("p (b e) -> p b e", e=E),
                    offs_b, ALU.add)
                kept = small.tile([P, NM * E], F32, tag="kept")
                nc.vector.tensor_scalar(
                    kept[:], locg[:], float(CAP) + 0.5, None, ALU.is_le)
                keptm = small.tile([P, NM * E], F32, tag="keptm")
                nc.vector.tensor_tensor(
                    keptm[:], kept[:], mask_all[:], ALU.mult)
                kflag = small.tile([P, NM], F32, tag="kflag")
                nc.vector.tensor_reduce(
                    kflag[:], keptm[:].rearrange("p (b e) -> p b e", e=E),
                    mybir.AxisListType.X, ALU.add)
                nc.vector.tensor_tensor(
                    scale_all[:], kflag[:], gate[:], ALU.mult)

            def scaled_evict(b, pm0, pm1):
                ot = outp.tile([P, D], F32, tag=f"ot{b}", name=f"ot{b}")
                st = outp.tile([P, D], F16, tag=f"st{b}", name=f"st{b}")
                sc = scale_all[:, b:b + 1]
                nc.vector.tensor_tensor(
                    ot[:, 0:HF], pm0[:], be_bc[:, 0:HF], ALU.add)
                nc.vector.tensor_scalar(
                    st[:, 0:HF], ot[:, 0:HF], sc, None, ALU.mult)
                nc.sync.dma_start(out_d[b * P:(b + 1) * P, 0:HF], st[:, 0:HF])
                nc.vector.tensor_tensor(
                    ot[:, HF:D], pm1[:], be_bc[:, HF:D], ALU.add)
                nc.scalar.activation(
                    st[:, HF:D], ot[:, HF:D], ACT_COPY, scale=sc)
                nc.sync.dma_start(out_d[b * P:(b + 1) * P, HF:D], st[:, HF:D])
                return ot

            scale_chain()
            for b in range(N_EARLY):
                store(b, early.pop(b))
            for b in range(N_EARLY, NM):
                pm0, pm1 = mm_tile(b)
                scaled_evict(b, pm0, pm1)

    nc.finalize()
    return nc


_NC_CACHE = None


def kernel(x: np.ndarray, Wg: np.ndarray, We: np.ndarray,
           be: np.ndarray) -> np.ndarray:
    global _NC_CACHE
    if _NC_CACHE is None:
        _NC_CACHE = _build_nc()
    nc = _NC_CACHE

    h = np.ascontiguousarray(np.asarray(x, dtype=np.float32).reshape(N, D))
    Wg = np.ascontiguousarray(np.asarray(Wg, dtype=np.float32))
    We = np.ascontiguousarray(np.asarray(We, dtype=np.float32))
    be2 = np.ascontiguousarray(np.asarray(be, dtype=np.float32).reshape(1, D))

    hT = np.ascontiguousarray(h.T)
    ht16 = hT.astype(np.float16)
    ht_lo = (hT - ht16.astype(np.float32)).astype(np.float16)
    Wg16 = Wg.astype(np.float16)
    Wg_loS = ((Wg - Wg16.astype(np.float32)) * 4096.0).astype(np.float16)
    Wgp16 = np.ascontiguousarray(np.concatenate([Wg16, Wg_loS], axis=1))
    We16 = We.astype(np.float16)

    in_maps = []
    for c in range(N_CORES):
        wpre = np.zeros((1, N_CORES), np.float32)
        wpre[0, :c] = 1.0
        in_maps.append({
            "htlo": np.ascontiguousarray(ht_lo[:, c * T:(c + 1) * T]),
            "wgp16": Wgp16,
            "ht16": np.ascontiguousarray(ht16[:, c * T:(c + 1) * T]),
            "we16": We16,
            "be": be2,
            "wpre": wpre,
        })

    res = run_bass_kernel_spmd(nc, in_maps, core_ids=list(range(N_CORES)))
    out = np.concatenate(
        [res.results[c]["out"] for c in range(N_CORES)], axis=0)
    return out.reshape(B, S, D).astype(np.float32)

